# revision 1
# baseline (speedup 1.0000x reference)
"""MetaConvSmoother Trainium2 kernel (Bass/Tile), data-parallel over 8 NeuronCores.

Per core (8 samples):
  - hypernet MLPs (9 -> 100 -> 147, exact gelu) on PE + ACT
  - per-sample conv kernels staged as zero-padded tables in DRAM
    (one 255-float table per (sample, tap-column); U[127-ky] = w[ky, kx])
  - flipped Toeplitz bands Bf[i, m] = U[i+m] loaded with all-positive strided
    DMAs, then partition-reversed on the TensorEngine with a constant
    anti-diagonal matrix: B[p, m] = U[127 + m - p]
  - each conv stage = banded matmuls over image rows (lhsT = B slices),
    column taps via free-dim offset reads of the rhs tile, PSUM accumulation:
      Ax   : 3x3, asymmetric pad (top/left 0, bottom/right 1.0)
      tmp_m: 7x7 corr of r = f - Ax        (3 maps)
      G2   : sum_m 7x7 corr of tmp_m
      out  = x + G2
  - r and tmp round-trip through DRAM to decouple row-tile alignments
  - DMA engine split to avoid FIFO head-of-line blocking:
      SP   : independent loads (x, f, bands)
      POOL : dependent loads (r-in, tmp-in) + table scatter writes (SWDGE)
      ACT  : stores (r-out, tmp-out, out) + band-reversal PSUM->SBUF copies
      DVE  : sub/add/copies/memsets
"""
import numpy as np

import concourse.bass as bass
import concourse.mybir as mybir
from concourse import bacc, bass_utils
from concourse.tile import TileContext

F32 = mybir.dt.float32
F32R = mybir.dt.float32r
USE_F32R = True


def _cast(ap):
    return ap
S = 8          # samples per core
N = 512
ML = 3
KK = 7
NCORES = 8

# table layout (elements) in the flat DRAM "tables" tensor
TBL = 255
BASE_A = 0                      # (s, kx)        -> 8*3 tables
BASE_S1 = 24 * TBL              # (s, m, kx)     -> 8*21
BASE_S2 = BASE_S1 + 168 * TBL
TBL_TOTAL = BASE_S2 + 168 * TBL  # 91800 elements

NSLOT = 45                      # band slots per sample: 3 A + 21 S1 + 21 S2
BANDW = 128                     # cols per band slot
BF = NSLOT * BANDW              # 5760

# row tilings (out_row_start, M, input_row_start)
AX_TILES = [(0, 126, -1), (126, 126, 125), (252, 126, 251), (378, 126, 377),
            (504, 8, 503)]
S7_TILES = [(0, 122, -3), (122, 122, 119), (244, 122, 241), (366, 122, 363),
            (488, 24, 485)]


def _sub_ap(base_ap, pattern, offset):
    """Custom access-pattern view: list of [step, count] pairs + elem offset."""
    a = base_ap.copy()
    v = a.ap
    v.clear()
    for p in pattern:
        v.append(list(p))
    a.offset = base_ap.offset + offset
    return a


def _slot_a(kx):
    return kx


def _slot_s1(m, kx):
    return 3 + m * KK + kx


def _slot_s2(m, kx):
    return 24 + m * KK + kx


def build_kernel(nc):
    x = nc.dram_tensor("x", [S, N, N], F32, kind="ExternalInput").ap()
    f = nc.dram_tensor("f", [S, N, N], F32, kind="ExternalInput").ap()
    ka = nc.dram_tensor("kernelA", [S, 9], F32, kind="ExternalInput").ap()
    fc_w1 = [nc.dram_tensor(f"fc{i}_w1", [100, 9], F32, kind="ExternalInput").ap()
             for i in (1, 2)]
    fc_b1 = [nc.dram_tensor(f"fc{i}_b1", [100], F32, kind="ExternalInput").ap()
             for i in (1, 2)]
    fc_w2 = [nc.dram_tensor(f"fc{i}_w2", [147, 100], F32, kind="ExternalInput").ap()
             for i in (1, 2)]
    fc_b2 = [nc.dram_tensor(f"fc{i}_b2", [147], F32, kind="ExternalInput").ap()
             for i in (1, 2)]
    out = nc.dram_tensor("out", [S, N, N], F32, kind="ExternalOutput").ap()

    with TileContext(nc) as tc:
        with (
            tc.tile_pool(name="dram", bufs=1, space="DRAM") as dpool,
            tc.tile_pool(name="const", bufs=1) as cpool,
            tc.tile_pool(name="mlp", bufs=1) as mpool,
            tc.tile_pool(name="bandf", bufs=1) as bfpool,
            tc.tile_pool(name="bands", bufs=2) as bpool,
            tc.tile_pool(name="xa", bufs=6) as xa_pool,
            tc.tile_pool(name="fr", bufs=4) as fr_pool,
            tc.tile_pool(name="rhs7", bufs=4) as rhs_pool,
            tc.tile_pool(name="stout", bufs=4) as st_pool,
            tc.tile_pool(name="psA", bufs=2, space="PSUM") as psA,
            tc.tile_pool(name="ps1", bufs=2, space="PSUM") as ps1,
            tc.tile_pool(name="ps2", bufs=2, space="PSUM") as ps2,
            tc.tile_pool(name="psx", bufs=2, space="PSUM") as psx,
        ):
            tables = dpool.tile([TBL_TOTAL], F32)
            r_dram = dpool.tile([S, N, N], F32)
            tmp_dram = dpool.tile([S, ML, N, N], F32)

            # ---- constants: anti-diagonal reversal matrix Rev[k,p]=d(k+p=127)
            rev = cpool.tile([128, 128], F32)
            nc.gpsimd.memset(rev, 0.0)
            nc.gpsimd.affine_select(
                out=rev, in_=rev, compare_op=mybir.AluOpType.not_equal,
                fill=1.0, base=-127, pattern=[[1, 128]], channel_multiplier=1)

            # ---- zero-fill tables
            zt = cpool.tile([120, 765], F32)
            nc.vector.memset(zt, 0.0)
            nc.sync.dma_start(_sub_ap(tables, [[765, 120], [1, 765]], 0), zt)

            # ---------------- MLP + weight staging ----------------
            ident = cpool.tile([128, 128], F32)
            nc.gpsimd.memset(ident, 0.0)
            nc.gpsimd.affine_select(
                out=ident, in_=ident, compare_op=mybir.AluOpType.not_equal,
                fill=1.0, base=0, pattern=[[-1, 128]], channel_multiplier=1)

            vT = mpool.tile([9, S], F32)
            nc.sync.dma_start(vT, ka.rearrange("s k -> k s"))

            w_sb = {}  # (layer i, map m) -> [49, S] conv weights
            for i in range(2):
                w1n = mpool.tile([100, 9], F32, name=f"w1n{i}")
                nc.sync.dma_start(w1n, fc_w1[i])
                W1T = mpool.tile([9, 100], F32, name=f"W1T{i}")
                t1 = psx.tile([9, 100], F32, name=f"t1_{i}", tag="aux")
                nc.tensor.transpose(t1, w1n, ident[:100, :100])
                nc.vector.tensor_copy(W1T, t1)

                b1 = mpool.tile([100, 1], F32, name=f"b1_{i}")
                nc.sync.dma_start(b1, fc_b1[i].unsqueeze(1))

                w2n_a = mpool.tile([128, 100], F32, name=f"w2na{i}")
                nc.sync.dma_start(w2n_a, fc_w2[i][0:128, :])
                w2n_b = mpool.tile([19, 100], F32, name=f"w2nb{i}")
                nc.sync.dma_start(w2n_b, fc_w2[i][128:147, :])
                W2T = mpool.tile([100, 147], F32, name=f"W2T{i}")
                tr_a = psx.tile([100, 128], F32, name=f"tra{i}", tag="aux")
                nc.tensor.transpose(tr_a, w2n_a, ident)
                nc.vector.tensor_copy(W2T[:, 0:128], tr_a)
                tr_b = psx.tile([100, 19], F32, name=f"trb{i}", tag="aux")
                nc.tensor.transpose(tr_b, w2n_b, ident[:19, :19])
                nc.vector.tensor_copy(W2T[:, 128:147], tr_b)

                h_pre = psx.tile([100, S], F32, name=f"hpre{i}", tag="aux")
                nc.tensor.matmul(h_pre, W1T, vT, start=True, stop=True)
                h = mpool.tile([100, S], F32, name=f"h{i}")
                nc.scalar.activation(
                    h, h_pre, mybir.ActivationFunctionType.Gelu, bias=b1)

                for m in range(ML):
                    b2m = mpool.tile([49, 1], F32, name=f"b2_{i}_{m}")
                    nc.sync.dma_start(
                        b2m, fc_b2[i][49 * m:49 * m + 49].unsqueeze(1))
                    wp = psx.tile([49, S], F32, name=f"wp{i}{m}", tag="aux")
                    nc.tensor.matmul(wp, W2T[:, 49 * m:49 * m + 49], h,
                                     start=True, stop=True)
                    wsb = mpool.tile([49, S], F32, name=f"w_{i}_{m}")
                    nc.scalar.activation(
                        wsb, wp, mybir.ActivationFunctionType.Identity,
                        bias=b2m)
                    w_sb[(i, m)] = wsb

            # scatter conv weights into zero-padded tables (SWDGE, flexible)
            # A tables: U[(s*3+kx)*255 + 127 - ky] = kernelA[s, ky, kx]
            for ky in range(3):
                nc.gpsimd.dma_start(
                    _sub_ap(tables, [[TBL, 3], [3 * TBL, S]],
                            BASE_A + 127 - ky),
                    vT[3 * ky:3 * ky + 3, :])
            # stage1/2: U[((s*3+m)*7+kx)*255 + 127 - ky] = w[i][s, m, ky, kx]
            for i, base in ((0, BASE_S1), (1, BASE_S2)):
                for m in range(ML):
                    for ky in range(KK):
                        nc.gpsimd.dma_start(
                            _sub_ap(tables, [[TBL, KK], [21 * TBL, S]],
                                    base + m * KK * TBL + 127 - ky),
                            w_sb[(i, m)][KK * ky:KK * ky + KK, :])

            # ---------------- main per-sample loop ----------------
            for s in range(S):
                # ---- flipped bands Bf[i, slot, m] = U_slot[i + m]
                bf = bfpool.tile([128, BF], F32, name=f"bf{s}", tag="bf")
                for (nslots, slot0, base) in (
                        (3, 0, BASE_A + s * 3 * TBL),
                        (21, 3, BASE_S1 + s * 21 * TBL),
                        (21, 24, BASE_S2 + s * 21 * TBL)):
                    nc.sync.dma_start(
                        _sub_ap(bf, [[BF, 128], [BANDW, nslots], [1, BANDW]],
                                slot0 * BANDW),
                        _sub_ap(tables, [[1, 128], [TBL, nslots], [1, BANDW]],
                                base))
                # ---- reverse partitions on PE: B[p] = Bf[127-p]
                bb = bpool.tile([128, BF], F32R if USE_F32R else F32, name=f"bb{s}", tag="bands")
                for c in range(0, BF, 512):
                    w = min(512, BF - c)
                    pr = psx.tile([128, 512], F32, name=f"pr{s}_{c}", tag="aux")
                    nc.tensor.matmul(pr[:, :w], rev, bf[:, c:c + w],
                                     start=True, stop=True)
                    nc.scalar.copy(bb[:, c:c + w], pr[:, :w])

                def band(slot, M):
                    return bb[:, slot * BANDW:slot * BANDW + M]

                # ---- Ax and r = f - Ax (126-row tiles) ----
                for (o0, M, row_start) in AX_TILES:
                    xt = xa_pool.tile([128, N + 2], F32R if USE_F32R else F32,
                                      name=f"xt{s}_{o0}", tag="xa")
                    if row_start + 128 > N:          # bottom tile: ones pad
                        nc.gpsimd.memset(xt.bitcast(F32), 1.0)
                        nd = N - row_start
                        nc.gpsimd.dma_start(xt[0:nd, 1:N + 1],
                                            x[s, row_start:N, :])
                        nc.gpsimd.memset(xt[0:nd, 0:1].bitcast(F32), 0.0)
                    else:
                        lo = max(0, row_start)
                        p0 = lo - row_start
                        if p0 > 0:
                            nc.gpsimd.memset(xt[0:p0, :].bitcast(F32), 0.0)
                        nc.gpsimd.dma_start(xt[p0:128, 1:N + 1],
                                            x[s, lo:row_start + 128, :])
                        nc.gpsimd.memset(xt[:, 0:1].bitcast(F32), 0.0)
                        nc.gpsimd.memset(xt[:, N + 1:N + 2].bitcast(F32), 1.0)
                    ps = psA.tile([M, N], F32, name=f"psA{s}_{o0}", tag="ax")
                    for kx in range(3):
                        nc.tensor.matmul(ps, _cast(band(_slot_a(kx), M)),
                                         _cast(xt[:, kx:kx + N]),
                                         start=(kx == 0), stop=(kx == 2))
                    ft = fr_pool.tile([126, N], F32, name=f"ft{s}_{o0}",
                                      tag="f")
                    nc.sync.dma_start(ft[:M, :], f[s, o0:o0 + M, :])
                    rt = fr_pool.tile([126, N], F32, name=f"rt{s}_{o0}",
                                      tag="r")
                    nc.vector.tensor_sub(rt[:M, :], ft[:M, :], ps)
                    nc.scalar.dma_start(r_dram[s, o0:o0 + M, :], rt[:M, :])

                # ---- stage 1: tmp_m = corr7(r, w1_m) ----
                for (o0, M, row_start) in S7_TILES:
                    rt7 = rhs_pool.tile([128, N + 6], F32R if USE_F32R else F32,
                                        name=f"rt7_{s}_{o0}", tag="rt7")
                    nc.gpsimd.memset(rt7.bitcast(F32), 0.0)
                    lo = max(0, row_start)
                    hi = min(N, row_start + 128)
                    nc.gpsimd.dma_start(
                        rt7[lo - row_start:hi - row_start, 3:N + 3],
                        r_dram[s, lo:hi, :])
                    tm3 = st_pool.tile([122, 3 * N], F32,
                                       name=f"tm3_{s}_{o0}", tag="tmp")
                    for m in range(ML):
                        ps = ps1.tile([M, N], F32, name=f"ps1_{s}_{o0}_{m}",
                                      tag="s1")
                        for kx in range(KK):
                            nc.tensor.matmul(ps, _cast(band(_slot_s1(m, kx), M)),
                                             _cast(rt7[:, kx:kx + N]),
                                             start=(kx == 0), stop=(kx == 6))
                        nc.vector.tensor_copy(tm3[:M, m * N:(m + 1) * N], ps)
                    # one store for all 3 maps: tmp_dram[s, :, o0:o0+M, :]
                    nc.scalar.dma_start(
                        _sub_ap(tmp_dram, [[N, M], [N * N, ML], [1, N]],
                                ((s * ML) * N + o0) * N),
                        _sub_ap(tm3, [[3 * N, M], [N, ML], [1, N]], 0))

                # ---- stage 2: G2 = sum_m corr7(tmp_m, w2_m); out = x + G2
                for (o0, M, row_start) in S7_TILES:
                    lo = max(0, row_start)
                    hi = min(N, row_start + 128)
                    tt = rhs_pool.tile([128, 3 * (N + 6)], F32R if USE_F32R else F32,
                                       name=f"tt{s}_{o0}", tag="tt")
                    nc.gpsimd.memset(tt.bitcast(F32), 0.0)
                    # one load for all 3 maps, each into its 518-block at col 3
                    nc.gpsimd.dma_start(
                        _sub_ap(tt, [[3 * (N + 6), hi - lo],
                                     [N + 6, ML], [1, N]],
                                (lo - row_start) * 3 * (N + 6) + 3),
                        _sub_ap(tmp_dram, [[N, hi - lo], [N * N, ML], [1, N]],
                                ((s * ML) * N + lo) * N))
                    pg = ps2.tile([M, N], F32, name=f"ps2_{s}_{o0}", tag="s2")
                    idx = 0
                    for m in range(ML):
                        for kx in range(KK):
                            nc.tensor.matmul(
                                pg, _cast(band(_slot_s2(m, kx), M)),
                                _cast(tt[:, m * (N + 6) + kx:m * (N + 6) + kx + N]),
                                start=(idx == 0), stop=(idx == 20))
                            idx += 1
                    x2 = fr_pool.tile([126, N], F32, name=f"x2_{s}_{o0}",
                                      tag="x2")
                    nc.sync.dma_start(x2[:M, :], x[s, o0:o0 + M, :])
                    ob = st_pool.tile([122, N], F32, name=f"ob{s}_{o0}",
                                      tag="ob")
                    nc.vector.tensor_add(ob[:M, :], x2[:M, :], pg)
                    nc.scalar.dma_start(out[s, o0:o0 + M, :], ob[:M, :])
    return nc


_CACHED = None


def _get_nc():
    global _CACHED
    if _CACHED is None:
        nc = bacc.Bacc("TRN2", debug=False, enable_asserts=False,
                       num_devices=NCORES)
        build_kernel(nc)
        nc.compile()
        _CACHED = nc
    return _CACHED


def make_in_maps(x, f, kernelA, fc1_w1, fc1_b1, fc1_w2, fc1_b2,
                 fc2_w1, fc2_b1, fc2_w2, fc2_b2):
    shared = {
        "fc1_w1": np.ascontiguousarray(fc1_w1, np.float32),
        "fc1_b1": np.ascontiguousarray(fc1_b1, np.float32),
        "fc1_w2": np.ascontiguousarray(fc1_w2, np.float32),
        "fc1_b2": np.ascontiguousarray(fc1_b2, np.float32),
        "fc2_w1": np.ascontiguousarray(fc2_w1, np.float32),
        "fc2_b1": np.ascontiguousarray(fc2_b1, np.float32),
        "fc2_w2": np.ascontiguousarray(fc2_w2, np.float32),
        "fc2_b2": np.ascontiguousarray(fc2_b2, np.float32),
    }
    in_maps = []
    for c in range(NCORES):
        sl = slice(S * c, S * (c + 1))
        in_maps.append({
            "x": np.ascontiguousarray(x[sl, 0], np.float32),
            "f": np.ascontiguousarray(f[sl, 0], np.float32),
            "kernelA": np.ascontiguousarray(
                kernelA[sl, 0].reshape(S, 9), np.float32),
            **shared,
        })
    return in_maps


def kernel(x, f, kernelA, fc1_w1, fc1_b1, fc1_w2, fc1_b2,
           fc2_w1, fc2_b1, fc2_w2, fc2_b2):
    x = np.asarray(x)
    nc = _get_nc()
    in_maps = make_in_maps(x, f, kernelA, fc1_w1, fc1_b1, fc1_w2, fc1_b2,
                           fc2_w1, fc2_b1, fc2_w2, fc2_b2)
    res = bass_utils.run_bass_kernel_spmd(
        nc, in_maps, core_ids=list(range(NCORES)))
    outs = [res.results[c]["out"] for c in range(NCORES)]
    full = np.concatenate(outs, axis=0).reshape(64, 1, N, N).astype(np.float32)
    return full



# revision 19
# speedup vs baseline: 1.5873x; 1.5873x over previous
"""MetaConvSmoother Trainium2 kernel (Bass/Tile), data-parallel over 8 cores.

Composed-conv design (validated index-for-index by model.py):
  - hypernet MLPs (9->100->147, exact gelu) on PE + ACT
  - the two 7x7 conv stages are composed on-chip into ONE 13x13 kernel
    per sample: W13 = sum_m fullconv2d(w1_m, w2_m), computed by a tiny
    banded correlation on the PE (w2 scattered flipped into tables,
    w1 placed in a 19-col canvas)
  - main pass: Ax (3x3, asymmetric pad) -> r = f - Ax -> DRAM (bf16);
    then G2' = corr13(r) + x, 5 row tiles of M=116 (48 last)
  - G2' differs from the reference two-stage result only on a 3-pixel
    border (stage-2 re-zeroing of the tmp frame); corrections are
    computed from frame strips of r with sample-batched block-diagonal
    banded matmuls and subtracted during assembly
  - bands built via zero-padded DRAM tables read with overlapping
    windows, partition-reversed on the PE with an anti-diagonal matrix
  - convolution data path in bf16 (PSUM fp32), W13 composition in f32r
"""
import os
from contextlib import ExitStack

import numpy as np
import ml_dtypes

import concourse.bass as bass
import concourse.mybir as mybir
from concourse import bacc, bass_utils
from concourse.tile import TileContext

F32 = mybir.dt.float32
F32R = mybir.dt.float32r
BF16 = mybir.dt.bfloat16

S = 8          # samples per core
N = 512
ML = 3
KK = 7
NCORES = 8
TBL = 255

NT = 5
MT = (116, 116, 116, 116, 48)
O0 = (0, 116, 232, 348, 464)
BW = 116       # band slot width

NPB = np.dtype(ml_dtypes.bfloat16)
DEBUG = os.environ.get('KDBG') == '1'


def _sub_ap(base_ap, pattern, offset):
    a = base_ap.copy()
    v = a.ap
    v.clear()
    for p in pattern:
        v.append(list(p))
    a.offset = base_ap.offset + offset
    return a


def build_kernel(nc):
    x = nc.dram_tensor("x", [S, N, N], BF16, kind="ExternalInput").ap()
    f = nc.dram_tensor("f", [S, N, N], BF16, kind="ExternalInput").ap()
    ka = nc.dram_tensor("kernelA", [S, 9], F32, kind="ExternalInput").ap()
    fc_w1 = [nc.dram_tensor(f"fc{i}_w1", [100, 9], F32, kind="ExternalInput").ap()
             for i in (1, 2)]
    fc_b1 = [nc.dram_tensor(f"fc{i}_b1", [100], F32, kind="ExternalInput").ap()
             for i in (1, 2)]
    fc_w2 = [nc.dram_tensor(f"fc{i}_w2", [147, 100], F32, kind="ExternalInput").ap()
             for i in (1, 2)]
    fc_b2 = [nc.dram_tensor(f"fc{i}_b2", [147], F32, kind="ExternalInput").ap()
             for i in (1, 2)]
    out = nc.dram_tensor("out", [S, N, N], F32, kind="ExternalOutput").ap()

    with TileContext(nc) as tc, ExitStack() as stk:
            dpool = stk.enter_context(tc.tile_pool(name="dram", bufs=1, space="DRAM"))
            cpool = stk.enter_context(tc.tile_pool(name="const", bufs=1))
            mpool = stk.enter_context(tc.tile_pool(name="mlp", bufs=1))
            xpool = stk.enter_context(tc.tile_pool(name="xa", bufs=4))
            xbpool = stk.enter_context(tc.tile_pool(name="xb", bufs=4))
            ctpool = stk.enter_context(tc.tile_pool(name="ctb", bufs=2))
            bkpool = stk.enter_context(tc.tile_pool(name="bandk", bufs=1))
            bfpool = stk.enter_context(tc.tile_pool(name="bf", bufs=2))
            typool = stk.enter_context(tc.tile_pool(name="tiny", bufs=2))
            fr_pool = stk.enter_context(tc.tile_pool(name="fr", bufs=4))
            rt_pool = stk.enter_context(tc.tile_pool(name="rt", bufs=4))
            rhs_pool = stk.enter_context(tc.tile_pool(name="rhs13", bufs=4))
            st_pool = stk.enter_context(tc.tile_pool(name="stout", bufs=4))
            bopool = stk.enter_context(tc.tile_pool(name="bord", bufs=1))
            psc = stk.enter_context(tc.tile_pool(name="psc", bufs=2, space="PSUM"))
            psx = stk.enter_context(tc.tile_pool(name="psx", bufs=2, space="PSUM"))
            psf = stk.enter_context(tc.tile_pool(name="psf", bufs=2, space="PSUM"))
            psf6 = stk.enter_context(tc.tile_pool(name="psf6", bufs=2, space="PSUM"))
            del stk
            # ---------------- DRAM tables ----------------
            tA = dpool.tile([S * 3 * TBL], F32)              # 6120 = 8*765
            tW = dpool.tile([36 * 765], F32)                 # >= 8*13*255
            tT = dpool.tile([S * 21 * TBL], F32)             # 42840 = 56*765
            t2 = dpool.tile([S * 3 * 49], F32)
            tS = dpool.tile([150 * 765], F32)                # >= 8*(8*7*255)
            r_dram = dpool.tile([S, N, N], BF16)

            # ---------------- constants ----------------
            rev0 = cpool.tile([128, 128], F32)
            nc.gpsimd.memset(rev0, 0.0)
            nc.gpsimd.affine_select(
                out=rev0, in_=rev0,
                compare_op=mybir.AluOpType.not_equal,
                fill=1.0, base=-127, pattern=[[1, 128]], channel_multiplier=1)
            rev = cpool.tile([128, 128], F32R)
            nc.scalar.copy(rev, rev0)

            ident = cpool.tile([128, 128], F32)
            nc.gpsimd.memset(ident, 0.0)
            nc.gpsimd.affine_select(
                out=ident, in_=ident, compare_op=mybir.AluOpType.not_equal,
                fill=1.0, base=0, pattern=[[-1, 128]], channel_multiplier=1)

            # zero-fill tables
            zt = cpool.tile([128, 765], F32)
            nc.vector.memset(zt, 0.0)

            def zfill(tile, nelem):
                assert nelem % 765 == 0
                k = nelem // 765
                o = 0
                while k > 0:
                    kk = min(k, 128)
                    nc.sync.dma_start(
                        _sub_ap(tile, [[765, kk], [1, 765]], o * 765),
                        zt[0:kk, :])
                    k -= kk
                    o += kk

            zfill(tA, S * 3 * TBL)
            zfill(tW, 36 * 765)
            zfill(tT, S * 21 * TBL)
            zfill(tS, 150 * 765)

            # ---------------- MLP ----------------
            vT = mpool.tile([9, S], F32)
            nc.sync.dma_start(vT, ka.rearrange("s k -> k s"))

            w_sb = {}  # (layer i, map m) -> [49, S]
            for i in range(2):
                w1n = mpool.tile([100, 9], F32, name=f"w1n{i}")
                nc.sync.dma_start(w1n, fc_w1[i])
                W1T = mpool.tile([9, 100], F32, name=f"W1T{i}")
                t1 = psx.tile([9, 100], F32, name=f"t1_{i}", tag="aux")
                nc.tensor.transpose(t1, w1n, ident[:100, :100])
                nc.vector.tensor_copy(W1T, t1)

                b1 = mpool.tile([100, 1], F32, name=f"b1_{i}")
                nc.sync.dma_start(b1, fc_b1[i].unsqueeze(1))

                w2n_a = mpool.tile([128, 100], F32, name=f"w2na{i}")
                nc.sync.dma_start(w2n_a, fc_w2[i][0:128, :])
                w2n_b = mpool.tile([19, 100], F32, name=f"w2nb{i}")
                nc.sync.dma_start(w2n_b, fc_w2[i][128:147, :])
                W2T = mpool.tile([100, 147], F32, name=f"W2T{i}")
                tr_a = psx.tile([100, 128], F32, name=f"tra{i}", tag="aux")
                nc.tensor.transpose(tr_a, w2n_a, ident)
                nc.vector.tensor_copy(W2T[:, 0:128], tr_a)
                tr_b = psx.tile([100, 19], F32, name=f"trb{i}", tag="aux")
                nc.tensor.transpose(tr_b, w2n_b, ident[:19, :19])
                nc.vector.tensor_copy(W2T[:, 128:147], tr_b)

                h_pre = psx.tile([100, S], F32, name=f"hpre{i}", tag="aux")
                nc.tensor.matmul(h_pre, W1T, vT, start=True, stop=True)
                h = mpool.tile([100, S], F32, name=f"h{i}")
                nc.scalar.activation(
                    h, h_pre, mybir.ActivationFunctionType.Gelu, bias=b1)

                for m in range(ML):
                    b2m = mpool.tile([49, 1], F32, name=f"b2_{i}_{m}")
                    nc.sync.dma_start(
                        b2m, fc_b2[i][49 * m:49 * m + 49].unsqueeze(1))
                    wp = psx.tile([49, S], F32, name=f"wp{i}{m}", tag="aux")
                    nc.tensor.matmul(wp, W2T[:, 49 * m:49 * m + 49], h,
                                     start=True, stop=True)
                    wsb = mpool.tile([49, S], F32, name=f"w_{i}_{m}")
                    nc.scalar.activation(
                        wsb, wp, mybir.ActivationFunctionType.Identity,
                        bias=b2m)
                    w_sb[(i, m)] = wsb

            # ---------------- scatters: A bands ----------------
            # tA[(s*3+kx)*TBL + 127-ky] = ka[s, ky*3+kx]
            for ky in range(3):
                nc.gpsimd.dma_start(
                    _sub_ap(tA, [[TBL, 3], [3 * TBL, S]], 127 - ky),
                    vT[3 * ky:3 * ky + 3, :])

            # ---------------- bbA bands (persistent, bf16) ----------------
            bbA = {}
            for s in range(S):
                bfA = bfpool.tile([128, 3 * BW], F32, name=f"bfA{s}", tag="bf")
                nc.sync.dma_start(
                    bfA,
                    _sub_ap(tA, [[1, 128], [TBL, 3], [1, BW]], s * 3 * TBL))
                pr = psx.tile([128, 3 * BW], F32, name=f"prA{s}", tag="aux")
                nc.tensor.matmul(pr, rev, bfA.bitcast(F32R),
                                 start=True, stop=True)
                bb = bkpool.tile([128, 3 * BW], BF16, name=f"bbA{s}",
                                 tag=f"bbA{s}")
                nc.scalar.copy(bb, pr)
                bbA[s] = bb

            # ---------------- tiny-conv scatters ----------------
            # tT[((s*3+m)*7+kx)*TBL + 121+ky] = w2[s,m,ky,kx]
            for m in range(ML):
                for ky in range(KK):
                    nc.gpsimd.dma_start(
                        _sub_ap(tT, [[TBL, KK], [21 * TBL, S]],
                                m * KK * TBL + 121 + ky),
                        w_sb[(1, m)][KK * ky:KK * ky + KK, :])
            # t2[(s*3+m)*49 + t] = w1[s,m,t]
            for m in range(ML):
                nc.gpsimd.dma_start(
                    _sub_ap(t2, [[1, 49], [3 * 49, S]], m * 49),
                    w_sb[(0, m)])

            # ---------------- tiny conv -> W13 ----------------
            w13_all = mpool.tile([13, 13 * S], F32)
            for s in range(S):
                canvas = typool.tile([128, 19], F32, name=f"cv{s}", tag="cv")
                nc.vector.memset(canvas, 0.0)
                for m in range(ML):
                    nc.sync.dma_start(
                        _sub_ap(canvas, [[19, 7], [1, 7]],
                                (19 * m + 6) * 19 + 6),
                        _sub_ap(t2, [[7, 7], [1, 7]], (s * 3 + m) * 49))
                bfT = typool.tile([128, 91], F32, name=f"bfT{s}", tag="bfT")
                nc.vector.memset(bfT, 0.0)
                for m in range(ML):
                    nc.sync.dma_start(
                        _sub_ap(bfT, [[91, 19], [13, KK], [1, 13]],
                                (109 - 19 * m) * 91),
                        _sub_ap(tT, [[1, 19], [TBL, KK], [1, 13]],
                                (s * 3 + m) * KK * TBL + 109))
                prT = psx.tile([128, 91], F32, name=f"prT{s}", tag="aux")
                nc.tensor.matmul(prT, rev0, bfT,
                                 start=True, stop=True)
                bbT = typool.tile([128, 91], F32, name=f"bbT{s}", tag="bbT")
                nc.scalar.copy(bbT, prT)
                psW = psx.tile([13, 13], F32, name=f"psW{s}", tag="aux")
                for j in range(KK):
                    nc.tensor.matmul(
                        psW, bbT[:, j * 13:(j + 1) * 13],
                        canvas[:, (6 - j):(6 - j) + 13],
                        start=(j == 0), stop=(j == 6))
                nc.vector.tensor_copy(w13_all[:, 13 * s:13 * s + 13], psW)

            # W13 scatter: tW[(s*13+kx)*TBL + 127-ky] = W13[s][ky,kx]
            for ky in range(13):
                nc.gpsimd.dma_start(
                    _sub_ap(tW, [[TBL, 1], [13 * TBL, S], [TBL, 13]],
                            127 - ky),
                    _sub_ap(w13_all, [[13 * S, 1], [13, S], [1, 13]],
                            ky * 13 * S))

            # ---------------- phase A: Ax, r ----------------
            for s in range(S):
                for u in range(NT):
                    o0, M = O0[u], MT[u]
                    rs = o0 - 1
                    xt = xpool.tile([128, 514], BF16, name=f"xt{s}_{u}",
                                    tag="xa")
                    if u == NT - 1:
                        nc.gpsimd.memset(xt, 1.0)
                        nd = N - rs
                        nc.sync.dma_start(xt[0:nd, 1:513], x[s, rs:N, :])
                        nc.gpsimd.memset(xt[0:nd, 0:1], 0.0)
                    else:
                        p0 = 1 if u == 0 else 0
                        if p0:
                            nc.gpsimd.memset(xt[0:p0, :], 0.0)
                        nc.sync.dma_start(xt[p0:118, 1:513],
                                          x[s, max(0, rs):rs + 118, :])
                        nc.gpsimd.memset(xt[:, 0:1], 0.0)
                        nc.gpsimd.memset(xt[:, 513:514], 1.0)
                    ps = psc.tile([M, N], F32, name=f"psA{s}_{u}", tag="conv")
                    for kx in range(3):
                        nc.tensor.matmul(ps, bbA[s][:, kx * BW:kx * BW + M],
                                         xt[:, kx:kx + N],
                                         start=(kx == 0), stop=(kx == 2))
                    ft = fr_pool.tile([116, N], BF16, name=f"ft{s}_{u}",
                                      tag="f")
                    nc.scalar.dma_start(ft[:M, :], f[s, o0:o0 + M, :])
                    rt = rt_pool.tile([116, N], BF16, name=f"rt{s}_{u}",
                                      tag="r")
                    nc.vector.tensor_sub(rt[:M, :], ft[:M, :], ps)
                    nc.scalar.dma_start(r_dram[s, o0:o0 + M, :], rt[:M, :])

            # ---------------- bbW bands (persistent, bf16) ----------------
            bbW = {}
            for s in range(S):
                bfW = bfpool.tile([128, 13 * BW], F32, name=f"bfW{s}",
                                  tag="bf")
                nc.sync.dma_start(
                    bfW,
                    _sub_ap(tW, [[1, 128], [TBL, 13], [1, BW]],
                            s * 13 * TBL))
                bb = bkpool.tile([128, 13 * BW], BF16, name=f"bbW{s}",
                                 tag=f"bbW{s}")
                for c in range(0, 13 * BW, 512):
                    w = min(512, 13 * BW - c)
                    pr = psx.tile([128, 512], F32, name=f"prW{s}_{c}",
                                  tag="aux")
                    nc.tensor.matmul(pr[:, :w], rev,
                                     bfW.bitcast(F32R)[:, c:c + w],
                                     start=True, stop=True)
                    nc.scalar.copy(bb[:, c:c + w], pr[:, :w])
                bbW[s] = bb

            # ---------------- border scatters ----------------
            SB = [k * S * 7 * TBL for k in range(8)]   # structure bases in tS

            def w1rows(m, row):   # w_sb[(0,m)] rows row*7 .. +7 (a-contig)
                return w_sb[(0, m)][row * KK:row * KK + KK, :]

            def w1cols(m, col):   # rows a*7+col over a (stride 7)
                return _sub_ap(w_sb[(0, m)], [[KK * S, KK], [1, S]], col * S)

            def w2rows(m, row):
                return w_sb[(1, m)][row * KK:row * KK + KK, :]

            def w2cols(m, col):
                return _sub_ap(w_sb[(1, m)], [[KK * S, KK], [1, S]], col * S)

            for m in range(ML):
                for d in (-2, -1, 0):
                    off = 127 + 5 * m - d
                    nc.gpsimd.dma_start(
                        _sub_ap(tS, [[TBL, KK], [KK * TBL, S]], SB[0] + off),
                        w1rows(m, d + 6))
                    nc.gpsimd.dma_start(
                        _sub_ap(tS, [[TBL, KK], [KK * TBL, S]], SB[2] + off),
                        w1cols(m, d + 6))
                for d in (0, 1, 2):
                    off = 127 + 5 * m - d
                    nc.gpsimd.dma_start(
                        _sub_ap(tS, [[TBL, KK], [KK * TBL, S]], SB[1] + off),
                        w1rows(m, d))
                    nc.gpsimd.dma_start(
                        _sub_ap(tS, [[TBL, KK], [KK * TBL, S]], SB[3] + off),
                        w1cols(m, d))
                for by in (0, 1, 2):
                    nc.gpsimd.dma_start(
                        _sub_ap(tS, [[TBL, KK], [KK * TBL, S]],
                                SB[4] + 127 - 5 * m - by),
                        w2rows(m, by))
                for by in (4, 5, 6):
                    nc.gpsimd.dma_start(
                        _sub_ap(tS, [[TBL, KK], [KK * TBL, S]],
                                SB[5] + 127 - 5 * m + 6 - by),
                        w2rows(m, by))
                for bx in (0, 1, 2):
                    nc.gpsimd.dma_start(
                        _sub_ap(tS, [[TBL, KK], [KK * TBL, S]],
                                SB[6] + 127 - 5 * m - bx),
                        w2cols(m, bx))
                for bx in (4, 5, 6):
                    nc.gpsimd.dma_start(
                        _sub_ap(tS, [[TBL, KK], [KK * TBL, S]],
                                SB[7] + 127 - 5 * m + 6 - bx),
                        w2cols(m, bx))

            # ---------------- border strip canvases ----------------
            RTt = bopool.tile([128, 524], BF16, name="RTt")
            RTb = bopool.tile([128, 524], BF16, name="RTb")
            RCl = bopool.tile([128, 524], BF16, name="RCl")
            RCr = bopool.tile([128, 524], BF16, name="RCr")
            for t in (RTt, RTb, RCl, RCr):
                nc.vector.memset(t, 0.0)
            for t, r0 in ((RTt, 0), (RTb, N - 3)):
                for ry in range(3):
                    nc.sync.dma_start(
                        _sub_ap(t, [[16 * 524, S], [1, 512]], ry * 524 + 6),
                        _sub_ap(r_dram, [[N * N, S], [1, 512]],
                                (r0 + ry) * N))
            for t, c0 in ((RCl, 0), (RCr, N - 3)):
                for s in range(S):
                    nc.sync.dma_start(
                        _sub_ap(t, [[524, 3], [1, 512]], 16 * s * 524 + 6),
                        _sub_ap(r_dram, [[1, 3], [N, 512]], s * N * N + c0))

            # frame row mask: 1 on valid rows 16s + {5m + ty3}, else 0
            ones_t = cpool.tile([128, 518], BF16)
            nc.vector.memset(ones_t, 1.0)
            mask518 = cpool.tile([128, 518], BF16)
            nc.vector.memset(mask518, 0.0)
            for m in range(ML):
                for t3 in range(3):
                    nc.sync.dma_start(
                        _sub_ap(mask518, [[16 * 518, S], [1, 518]],
                                (5 * m + t3) * 518),
                        _sub_ap(ones_t, [[518, S], [1, 518]], 0))

            # ---------------- border structure bands ----------------
            BS = []
            for k in range(8):
                if k < 4:
                    wper, i0, ob, oc = 125, 13, 16, 13
                else:
                    wper, i0, ob, oc = 24, 3, 3, 3
                wpad = 7 * wper + (7 * wper) % 2
                bfS = bfpool.tile([128, wpad], F32, name=f"bfS{k}",
                                  tag="bf")
                nc.vector.memset(bfS, 0.0)
                for a in range(KK):
                    nc.sync.dma_start(
                        _sub_ap(bfS, [[wpad, 128 - i0], [ob, S], [1, oc]],
                                i0 * wpad + a * wper),
                        _sub_ap(tS, [[1, 128 - i0], [KK * TBL + 16, S],
                                     [1, oc]],
                                SB[k] + a * TBL + i0))
                bsk = bopool.tile([128, wpad], BF16, name=f"BS{k}")
                for c in range(0, wpad, 512):
                    w = min(512, wpad - c)
                    pr = psx.tile([128, 512], F32, name=f"prS{k}_{c}",
                                  tag="aux")
                    nc.tensor.matmul(pr[:, :w], rev,
                                     bfS.bitcast(F32R)[:, c:c + w],
                                     start=True, stop=True)
                    nc.scalar.copy(bsk[:, c:c + w], pr[:, :w])
                BS.append(bsk)

            # ---------------- frames ----------------
            def frame(k, rc, name, trim_cols):
                fcv = bopool.tile([128, 518], BF16, name=name)
                nc.vector.memset(fcv, 0.0)
                pF = psf.tile([125, 512], F32, name=f"pF{k}", tag="frame")
                pG = psf6.tile([125, 6], F32, name=f"pG{k}", tag="frame6")
                for a in range(KK):
                    nc.tensor.matmul(pF, BS[k][:, a * 125:a * 125 + 125],
                                     rc[:, a:a + 512],
                                     start=(a == 0), stop=(a == 6))
                for a in range(KK):
                    nc.tensor.matmul(pG, BS[k][:, a * 125:a * 125 + 125],
                                     rc[:, a + 512:a + 518],
                                     start=(a == 0), stop=(a == 6))
                if trim_cols:
                    nc.scalar.copy(fcv[0:125, 3:512], pF[:, 3:512])
                    nc.scalar.copy(fcv[0:125, 512:515], pG[:, 0:3])
                else:
                    nc.scalar.copy(fcv[0:125, 0:512], pF)
                    nc.scalar.copy(fcv[0:125, 512:518], pG)
                # zero hole/junk rows (valid rows: 16s + {5m+ty3})
                nc.vector.tensor_mul(fcv, fcv, mask518)
                return fcv

            TFc = frame(0, RTt, "TFc", False)
            TBc = frame(1, RTb, "TBc", False)
            LFl = frame(2, RCl, "LFl", True)
            LFr = frame(3, RCr, "LFr", True)

            # ---------------- corrections ----------------
            def corr(k, fcv, name):
                pC = psf.tile([24, 512], F32, name=f"pC{k}", tag="frame")
                for a in range(KK):
                    nc.tensor.matmul(pC, BS[k][:, a * 24:a * 24 + 24],
                                     fcv[:, a:a + 512],
                                     start=(a == 0), stop=(a == 6))
                cs = bopool.tile([24, 512], F32, name=name)
                nc.vector.tensor_copy(cs, pC)
                return cs

            Ctop = corr(4, TFc, "Ctop")
            Cbot = corr(5, TBc, "Cbot")
            ClT = corr(6, LFl, "ClT")
            CrT = corr(7, LFr, "CrT")

            # transpose col-corrections into per-tile [M, 24] chunks
            clch, crch = {}, {}
            for u in range(NT):
                o0, M = O0[u], MT[u]
                for csrc, dst, nm in ((ClT, clch, "l"), (CrT, crch, "r")):
                    pt = psx.tile([116, 24], F32, name=f"pt{nm}{u}",
                                  tag="aux")
                    nc.tensor.transpose(pt[:M, :], csrc[:, o0:o0 + M],
                                        ident[:24, :24])
                    ch = bopool.tile([116, 24], F32, name=f"ch{nm}{u}")
                    nc.vector.tensor_copy(ch[:M, :], pt[:M, :])
                    dst[u] = ch

            if DEBUG:
                dmk = nc.dram_tensor("dmask", [128, 518], F32,
                                     kind="ExternalOutput").ap()
                cvm = bopool.tile([128, 518], F32, name="cvmask")
                nc.vector.tensor_copy(cvm, mask518)
                nc.sync.dma_start(dmk, cvm)
                for nm, t in (("dCtop", Ctop), ("dCbot", Cbot),
                              ("dClT", ClT), ("dCrT", CrT)):
                    d = nc.dram_tensor(nm, [24, 512], F32,
                                       kind="ExternalOutput").ap()
                    nc.sync.dma_start(d, t)
                for nm, t in (("dTFc", TFc), ("dTBc", TBc),
                              ("dLFl", LFl), ("dLFr", LFr)):
                    d = nc.dram_tensor(nm, [128, 518], F32,
                                       kind="ExternalOutput").ap()
                    cv = bopool.tile([128, 518], F32, name=f"cv{nm}")
                    nc.vector.tensor_copy(cv, t)
                    nc.sync.dma_start(d, cv)
                for nm, t in (("dRTt", RTt), ("dRTb", RTb),
                              ("dRCl", RCl), ("dRCr", RCr)):
                    d = nc.dram_tensor(nm, [128, 524], F32,
                                       kind="ExternalOutput").ap()
                    cv = bopool.tile([128, 524], F32, name=f"cv{nm}")
                    nc.vector.tensor_copy(cv, t)
                    nc.sync.dma_start(d, cv)
                for k in (0, 4):
                    d = nc.dram_tensor(f"dBS{k}", [128, 876 if k < 4 else 168],
                                       F32, kind="ExternalOutput").ap()
                    cv = bopool.tile([128, 876 if k < 4 else 168], F32,
                                     name=f"cvBS{k}")
                    nc.vector.tensor_copy(cv, BS[k])
                    nc.sync.dma_start(d, cv)
                dts = nc.dram_tensor("dtS", [2 * S * KK * TBL], F32,
                                     kind="ExternalOutput").ap()
                nc.sync.dma_start(
                    _sub_ap(dts, [[1, S * KK * TBL]], 0),
                    _sub_ap(tS, [[1, S * KK * TBL]], SB[0]))
                nc.sync.dma_start(
                    _sub_ap(dts, [[1, S * KK * TBL]], S * KK * TBL),
                    _sub_ap(tS, [[1, S * KK * TBL]], SB[4]))

            # ---------------- phase B: 13x13 conv + assembly ----------------
            for s in range(S):
                for u in range(NT):
                    o0, M = O0[u], MT[u]
                    rt13 = rhs_pool.tile([128, 524], BF16,
                                         name=f"r13_{s}_{u}", tag="r13")
                    lo, hi = max(0, o0 - 6), min(N, o0 + 122)
                    plo, phi = lo - (o0 - 6), hi - (o0 - 6)
                    if plo > 0:
                        nc.gpsimd.memset(rt13[0:plo, :], 0.0)
                    if phi < 128:
                        nc.gpsimd.memset(rt13[32:64, :], 0.0)
                        nc.gpsimd.memset(rt13[64:96, :], 0.0)
                        nc.gpsimd.memset(rt13[96:128, :], 0.0)
                    nc.gpsimd.memset(rt13[:, 0:6], 0.0)
                    nc.gpsimd.memset(rt13[:, 518:524], 0.0)
                    nc.gpsimd.dma_start(rt13[plo:phi, 6:518],
                                        r_dram[s, lo:hi, :])
                    ps = psc.tile([M, N], F32, name=f"ps13_{s}_{u}",
                                  tag="conv")
                    for kx in range(13):
                        nc.tensor.matmul(ps, bbW[s][:, kx * BW:kx * BW + M],
                                         rt13[:, kx:kx + N],
                                         start=(kx == 0), stop=(kx == 12))
                    xb = xbpool.tile([116, N], BF16, name=f"xb{s}_{u}",
                                     tag="xb")
                    nc.sync.dma_start(xb[:M, :], x[s, o0:o0 + M, :])
                    ob = st_pool.tile([116, N], F32, name=f"ob{s}_{u}",
                                      tag="ob")
                    nc.vector.tensor_add(ob[:M, :], xb[:M, :], ps)
                    if u == 0:
                        ctp = ctpool.tile([3, N], F32, name=f"ctp{s}",
                                          tag="ctp")
                        nc.sync.dma_start(ctp, Ctop[3 * s:3 * s + 3, :])
                        nc.vector.tensor_sub(ob[0:3, :], ob[0:3, :], ctp)
                    if u == NT - 1:
                        cbp = ctpool.tile([48, N], F32, name=f"cbp{s}",
                                          tag="cbp")
                        nc.vector.memset(cbp[32:48, :], 0.0)
                        nc.sync.dma_start(cbp[45:48, :],
                                          Cbot[3 * s:3 * s + 3, :])
                        nc.vector.tensor_sub(ob[32:M, :], ob[32:M, :],
                                             cbp[32:M, :])
                    nc.vector.tensor_sub(ob[:M, 0:3], ob[:M, 0:3],
                                         clch[u][:M, 3 * s:3 * s + 3])
                    nc.vector.tensor_sub(ob[:M, 509:512], ob[:M, 509:512],
                                         crch[u][:M, 3 * s:3 * s + 3])
                    nc.scalar.dma_start(out[s, o0:o0 + M, :], ob[:M, :])
    return nc


_CACHED = None


def _get_nc():
    global _CACHED
    if _CACHED is None:
        nc = bacc.Bacc("TRN2", debug=False, enable_asserts=False,
                       num_devices=NCORES)
        build_kernel(nc)
        nc.compile()
        _CACHED = nc
    return _CACHED


def make_in_maps(x, f, kernelA, fc1_w1, fc1_b1, fc1_w2, fc1_b2,
                 fc2_w1, fc2_b1, fc2_w2, fc2_b2):
    shared = {
        "fc1_w1": np.ascontiguousarray(fc1_w1, np.float32),
        "fc1_b1": np.ascontiguousarray(fc1_b1, np.float32),
        "fc1_w2": np.ascontiguousarray(fc1_w2, np.float32),
        "fc1_b2": np.ascontiguousarray(fc1_b2, np.float32),
        "fc2_w1": np.ascontiguousarray(fc2_w1, np.float32),
        "fc2_b1": np.ascontiguousarray(fc2_b1, np.float32),
        "fc2_w2": np.ascontiguousarray(fc2_w2, np.float32),
        "fc2_b2": np.ascontiguousarray(fc2_b2, np.float32),
    }
    xb = np.asarray(x).astype(NPB)
    fb = np.asarray(f).astype(NPB)
    in_maps = []
    for c in range(NCORES):
        sl = slice(S * c, S * (c + 1))
        in_maps.append({
            "x": np.ascontiguousarray(xb[sl, 0]),
            "f": np.ascontiguousarray(fb[sl, 0]),
            "kernelA": np.ascontiguousarray(
                np.asarray(kernelA)[sl, 0].reshape(S, 9), np.float32),
            **shared,
        })
    return in_maps


def kernel(x, f, kernelA, fc1_w1, fc1_b1, fc1_w2, fc1_b2,
           fc2_w1, fc2_b1, fc2_w2, fc2_b2):
    x = np.asarray(x)
    nc = _get_nc()
    in_maps = make_in_maps(x, f, kernelA, fc1_w1, fc1_b1, fc1_w2, fc1_b2,
                           fc2_w1, fc2_b1, fc2_w2, fc2_b2)
    res = bass_utils.run_bass_kernel_spmd(
        nc, in_maps, core_ids=list(range(NCORES)))
    outs = [res.results[c]["out"] for c in range(NCORES)]
    full = np.concatenate(outs, axis=0).reshape(64, 1, N, N).astype(np.float32)
    return full
